# revision 29
# baseline (speedup 1.0000x reference)
"""Depthwise Conv1d (C=512, K=3, stride=1, pad=1) on 8 Trainium2 NeuronCores.

Problem: x [16, 512, 4096] f32, w [512, 1, 3] f32, b [512] f32
         out[n,c,l] = sum_k w[c,0,k] * x_pad[n,c,l+k] + b[c]

Correctness gate is rel_err < 2e-2; fp16 I/O keeps the L2 rel err at
~3.6e-4 while halving HBM traffic.

Sharding: (channel-block, batch-group) — core c handles channel block
c%4 (128 channels) for 8 of the 16 batches, i.e. 8 row-tiles of
[128, 4096]. One channel block per core means only 3 diagonal weight
matrices are needed for the whole kernel.

Per row-tile (fp16, zero-padded to [128, 4098] on the host):
  - loads in 0.5 MB halves (sync-engine HWDGE ring); the first tile in
    finer slices so the first matmul starts ~2us after loop entry
  - conv on TensorE: per 512-col chunk, 3 matmuls with stationary
    diag(w_k) [128,128] fp16 and moving xp[:, c*512+k : +512],
    accumulated into one PSUM bank (fp32); issue rate is the 216 ns
    N=512 roofline
  - evict PSUM -> SBUF fp16 with per-partition bias add, alternating
    VectorE tensor_scalar / ScalarE activation
  - stores as one 1 MB transfer per row-tile (scalar-engine HWDGE
    ring); the last tile stores in 4 slices, the final two on the by
    then idle sync ring, to shorten the tail
"""

import numpy as np

B, C, L, K = 16, 512, 4096, 3
N_CORES = 8
NBLK = 4                     # channel blocks of 128
B_SH = 8                     # batches per core
NT = B_SH                    # row-tiles per core (one channel block each)
CH = 512                     # matmul chunk columns (1 PSUM bank; fp16 ISA max)
NCH = L // CH
HALF = L // 2
LP = L + 2                   # padded row length

_STATE = {}


def _build_program():
    from contextlib import ExitStack

    import concourse.bacc as bacc
    import concourse.mybir as mybir
    import concourse.tile as tile

    f16 = mybir.dt.float16
    f32 = mybir.dt.float32
    nc = bacc.Bacc(
        "TRN2",
        target_bir_lowering=False,
        debug=False,
        num_devices=N_CORES,
    )
    x_d = nc.dram_tensor("x", [NT, 128, LP], f16, kind="ExternalInput").ap()
    wd_d = nc.dram_tensor("wd", [128, 3 * 128], f16, kind="ExternalInput").ap()
    w1_d = nc.dram_tensor("w1col", [128, 1], f32, kind="ExternalInput").ap()
    bias_d = nc.dram_tensor("bias", [128, 1], f32, kind="ExternalInput").ap()
    o_d = nc.dram_tensor("out", [NT, 128, L], f16, kind="ExternalOutput").ap()

    with tile.TileContext(nc) as tc, ExitStack() as ctx:
        wpool = ctx.enter_context(tc.tile_pool(name="wpool", bufs=1))
        xpool = ctx.enter_context(tc.tile_pool(name="xpool", bufs=4))
        opool = ctx.enter_context(tc.tile_pool(name="opool", bufs=3))
        ppool = ctx.enter_context(
            tc.tile_pool(name="ppool", bufs=7, space="PSUM")
        )
        upool = ctx.enter_context(tc.tile_pool(name="upool", bufs=3))

        # chunks where PE does only taps 0+2 (2 passes instead of 3);
        # ACT computes tap1+bias into a scratch and one DVE tensor_tensor
        # fuses combine+evict. Shifts ~5us of PE streaming onto the
        # slack DVE/ACT capacity.
        C_CHUNKS = {(t, c) for t in range(NT) for c in (0, 2, 5)}

        wd = wpool.tile([128, 3 * 128], f16)
        w1c = wpool.tile([128, 1], f32)
        bias = wpool.tile([128, 1], f32)
        # weights on the store (scalar) ring, which is idle at start, so
        # the x loads own the sync ring from t=0
        nc.scalar.dma_start(wd[:, :], wd_d)
        nc.scalar.dma_start(w1c[:, :], w1_d)
        nc.scalar.dma_start(bias[:, :], bias_d)

        # PE is idle during the load ramp; dummy matmuls on a zeroed
        # scratch tile warm the HAM clock gate (4/8 -> 8/8) so the real
        # stream runs at 2.4 GHz from its first group
        warm = wpool.tile([128, CH], f16)
        nc.vector.memset(warm[:, :], 0.0)
        wps = ppool.tile([128, CH], f32, tag="warm", bufs=1)
        for _ in range(8):
            nc.tensor.matmul(
                wps[:, :], warm[:, 0:128], warm[:, :], start=True, stop=True
            )

        pending_sync_store = None
        for t in range(NT):
            xp = xpool.tile([128, LP], f16, tag="xp")
            if t == 0:
                # finer first loads so the first matmul starts ASAP; cuts
                # cover each chunk's +2 column overhang
                cuts = [0, 578, 1090, 2114, 3138, LP]
            else:
                cuts = [0, HALF + 1, LP]
            for a, b in zip(cuts, cuts[1:]):
                nc.sync.dma_start(xp[:, a:b], x_d[t][:, a:b])
            if pending_sync_store is not None:
                # tile 6's store rides the sync ring, emitted after the
                # final load triggers so it can't head-of-line block them;
                # splitting the trailing ~2.3 MB of stores across both
                # rings drains the tail sooner
                dst, src = pending_sync_store
                nc.sync.dma_start(dst, src)
                pending_sync_store = None

            last = t == NT - 1
            ot = opool.tile([128, L], f16, tag="ot")
            for c in range(NCH):
                oc = ot[:, c * CH : (c + 1) * CH]
                ps = ppool.tile([128, CH], f32, tag="ps")
                if (t, c) in C_CHUNKS:
                    tb = upool.tile([128, CH], f16, tag="tb")
                    nc.scalar.activation(
                        tb,
                        xp[:, c * CH + 1 : c * CH + CH + 1],
                        mybir.ActivationFunctionType.Identity,
                        bias=bias[:, 0:1],
                        scale=w1c[:, 0:1],
                    )
                    for k in (0, 2):
                        nc.tensor.matmul(
                            ps[:, :],
                            wd[:, k * 128 : (k + 1) * 128],
                            xp[:, c * CH + k : c * CH + k + CH],
                            start=(k == 0),
                            stop=(k == 2),
                        )
                    nc.vector.tensor_tensor(
                        oc, ps[:, :], tb[:, :], mybir.AluOpType.add
                    )
                else:
                    for k in range(3):
                        nc.tensor.matmul(
                            ps[:, :],
                            wd[:, k * 128 : (k + 1) * 128],
                            xp[:, c * CH + k : c * CH + k + CH],
                            start=(k == 0),
                            stop=(k == 2),
                        )
                    if c % 2 == 1:
                        nc.vector.tensor_scalar(
                            oc, ps[:, :], bias[:, 0:1], None, mybir.AluOpType.add
                        )
                    else:
                        nc.scalar.activation(
                            oc,
                            ps[:, :],
                            mybir.ActivationFunctionType.Identity,
                            bias=bias[:, 0:1],
                            scale=1.0,
                        )
                if not last:
                    continue
                # last row-tile: fine-grained stores, final two on the
                # (now idle) sync ring, to shorten the tail
                if c == 3:
                    nc.scalar.dma_start(o_d[t][:, 0:HALF], ot[:, 0:HALF])
                elif c == 5:
                    nc.scalar.dma_start(o_d[t][:, HALF : 6 * CH], ot[:, HALF : 6 * CH])
                elif c == 6:
                    nc.sync.dma_start(
                        o_d[t][:, 6 * CH : 7 * CH], ot[:, 6 * CH : 7 * CH]
                    )
                elif c == 7:
                    nc.sync.dma_start(o_d[t][:, 7 * CH : L], ot[:, 7 * CH : L])
            if not last:
                if t == NT - 2:
                    pending_sync_store = (o_d[t], ot[:, :])
                else:
                    nc.scalar.dma_start(o_d[t], ot[:, :])

    nc.compile()
    return nc


def _pack_weights(w, b):
    """Per channel block: 3 diag [128,128] f16 stacked -> [128, 384], plus
    the f32 bias column [128, 1]."""
    w = np.asarray(w, dtype=np.float32).reshape(C, K)
    b = np.asarray(b, dtype=np.float32)
    wds, w1s, biases = [], [], []
    idx = np.arange(128)
    for blk in range(NBLK):
        wblk = w[blk * 128 : (blk + 1) * 128]
        wd = np.zeros((128, 3 * 128), np.float16)
        for k in range(3):
            wd[idx, k * 128 + idx] = wblk[:, k].astype(np.float16)
        wds.append(wd)
        w1s.append(np.ascontiguousarray(wblk[:, 1].reshape(128, 1)))
        biases.append(b[blk * 128 : (blk + 1) * 128].reshape(128, 1))
    return wds, w1s, biases


def _run(inputs, trace=False, **kw):
    from concourse.bass_utils import run_bass_kernel_spmd

    if "nc" not in _STATE:
        _STATE["nc"] = _build_program()
    nc = _STATE["nc"]

    x = np.asarray(inputs["x"], dtype=np.float32).astype(np.float16)
    xpad = np.zeros((B, C, LP), np.float16)
    xpad[:, :, 1 : L + 1] = x
    wds, w1s, biases = _pack_weights(inputs["w"], inputs["b"])
    in_maps = []
    for core in range(N_CORES):
        blk = core % NBLK
        g = core // NBLK
        shard = np.ascontiguousarray(
            xpad[g * B_SH : (g + 1) * B_SH, blk * 128 : (blk + 1) * 128, :]
        )
        in_maps.append(
            {"x": shard, "wd": wds[blk], "w1col": w1s[blk], "bias": biases[blk]}
        )
    res = run_bass_kernel_spmd(
        nc, in_maps, core_ids=list(range(N_CORES)), trace=trace, **kw
    )
    out = np.empty((B, C, L), np.float32)
    for core in range(N_CORES):
        blk = core % NBLK
        g = core // NBLK
        out[g * B_SH : (g + 1) * B_SH, blk * 128 : (blk + 1) * 128, :] = res.results[
            core
        ]["out"].astype(np.float32)
    return out, res


def kernel(**inputs):
    return _run(inputs)[0]


# revision 32
# speedup vs baseline: 1.0243x; 1.0243x over previous
"""Depthwise Conv1d (C=512, K=3, stride=1, pad=1) on 8 Trainium2 NeuronCores.

Problem: x [16, 512, 4096] f32, w [512, 1, 3] f32, b [512] f32
         out[n,c,l] = sum_k w[c,0,k] * x_pad[n,c,l+k] + b[c]

Correctness gate is rel_err < 2e-2; fp16 I/O keeps the L2 rel err at
~3.6e-4 while halving HBM traffic.

Sharding: (channel-block, batch-group) — core c handles channel block
c%4 (128 channels) for 8 of the 16 batches, i.e. 8 row-tiles of
[128, 4096]. One channel block per core means only 3 diagonal weight
matrices are needed for the whole kernel.

Per row-tile (fp16, zero-padded to [128, 4098] on the host):
  - loads in 0.5 MB halves (sync-engine HWDGE ring); the first tile in
    finer slices so the first matmul starts ~2us after loop entry
  - conv on TensorE: per 512-col chunk, 3 matmuls with stationary
    diag(w_k) [128,128] fp16 and moving xp[:, c*512+k : +512],
    accumulated into one PSUM bank (fp32); issue rate is the 216 ns
    N=512 roofline
  - evict PSUM -> SBUF fp16 with per-partition bias add, alternating
    VectorE tensor_scalar / ScalarE activation
  - stores as one 1 MB transfer per row-tile (scalar-engine HWDGE
    ring); the last tile stores in 4 slices, the final two on the by
    then idle sync ring, to shorten the tail
"""

import numpy as np

B, C, L, K = 16, 512, 4096, 3
N_CORES = 8
NBLK = 4                     # channel blocks of 128
B_SH = 8                     # batches per core
NT = B_SH                    # row-tiles per core (one channel block each)
CH = 512                     # matmul chunk columns (1 PSUM bank; fp16 ISA max)
NCH = L // CH
HALF = L // 2
LP = L + 2                   # padded row length

_STATE = {}


def _build_program():
    from contextlib import ExitStack

    import concourse.bacc as bacc
    import concourse.mybir as mybir
    import concourse.tile as tile

    f16 = mybir.dt.float16
    f32 = mybir.dt.float32
    nc = bacc.Bacc(
        "TRN2",
        target_bir_lowering=False,
        debug=False,
        num_devices=N_CORES,
    )
    x_d = nc.dram_tensor("x", [NT, 128, LP], f16, kind="ExternalInput").ap()
    wd_d = nc.dram_tensor("wd", [128, 3 * 128], f16, kind="ExternalInput").ap()
    w1_d = nc.dram_tensor("w1col", [128, 1], f32, kind="ExternalInput").ap()
    bias_d = nc.dram_tensor("bias", [128, 1], f32, kind="ExternalInput").ap()
    o_d = nc.dram_tensor("out", [NT, 128, L], f16, kind="ExternalOutput").ap()

    with tile.TileContext(nc) as tc, ExitStack() as ctx:
        wpool = ctx.enter_context(tc.tile_pool(name="wpool", bufs=1))
        xpool = ctx.enter_context(tc.tile_pool(name="xpool", bufs=4))
        opool = ctx.enter_context(tc.tile_pool(name="opool", bufs=3))
        ppool = ctx.enter_context(
            tc.tile_pool(name="ppool", bufs=7, space="PSUM")
        )
        upool = ctx.enter_context(tc.tile_pool(name="upool", bufs=3))

        # chunks where PE does only taps 0+2 (2 passes instead of 3);
        # ACT computes tap1+bias into a scratch and one DVE tensor_tensor
        # fuses combine+evict. Measured: 3 per tile saturates DVE and
        # stalls the PE on PSUM release (64.0us vs 59.8us); 1 per tile
        # keeps DVE at ~58% while trimming the PE stream ~1.7us.
        C_CHUNKS = {(t, 4) for t in range(NT)}

        wd = wpool.tile([128, 3 * 128], f16)
        w1c = wpool.tile([128, 1], f32)
        bias = wpool.tile([128, 1], f32)
        # weights on the store (scalar) ring, which is idle at start, so
        # the x loads own the sync ring from t=0
        nc.scalar.dma_start(wd[:, :], wd_d)
        nc.scalar.dma_start(w1c[:, :], w1_d)
        nc.scalar.dma_start(bias[:, :], bias_d)

        # PE is idle during the load ramp; dummy matmuls on a zeroed
        # scratch tile warm the HAM clock gate (4/8 -> 8/8) so the real
        # stream runs at 2.4 GHz from its first group
        warm = wpool.tile([128, CH], f16)
        nc.vector.memset(warm[:, :], 0.0)
        wps = ppool.tile([128, CH], f32, tag="warm", bufs=1)
        for _ in range(8):
            nc.tensor.matmul(
                wps[:, :], warm[:, 0:128], warm[:, :], start=True, stop=True
            )

        pending_sync_store = None
        for t in range(NT):
            xp = xpool.tile([128, LP], f16, tag="xp")
            if t == 0:
                # finer first loads so the first matmul starts ASAP; cuts
                # cover each chunk's +2 column overhang
                cuts = [0, 578, 1090, 2114, 3138, LP]
            else:
                cuts = [0, HALF + 1, LP]
            for a, b in zip(cuts, cuts[1:]):
                nc.sync.dma_start(xp[:, a:b], x_d[t][:, a:b])
            if pending_sync_store is not None:
                # tile 6's store rides the sync ring, emitted after the
                # final load triggers so it can't head-of-line block them;
                # splitting the trailing ~2.3 MB of stores across both
                # rings drains the tail sooner
                dst, src = pending_sync_store
                nc.sync.dma_start(dst, src)
                pending_sync_store = None

            last = t == NT - 1
            ot = opool.tile([128, L], f16, tag="ot")
            for c in range(NCH):
                oc = ot[:, c * CH : (c + 1) * CH]
                ps = ppool.tile([128, CH], f32, tag="ps")
                if (t, c) in C_CHUNKS:
                    tb = upool.tile([128, CH], f16, tag="tb")
                    nc.scalar.activation(
                        tb,
                        xp[:, c * CH + 1 : c * CH + CH + 1],
                        mybir.ActivationFunctionType.Identity,
                        bias=bias[:, 0:1],
                        scale=w1c[:, 0:1],
                    )
                    for k in (0, 2):
                        nc.tensor.matmul(
                            ps[:, :],
                            wd[:, k * 128 : (k + 1) * 128],
                            xp[:, c * CH + k : c * CH + k + CH],
                            start=(k == 0),
                            stop=(k == 2),
                        )
                    nc.vector.tensor_tensor(
                        oc, ps[:, :], tb[:, :], mybir.AluOpType.add
                    )
                else:
                    for k in range(3):
                        nc.tensor.matmul(
                            ps[:, :],
                            wd[:, k * 128 : (k + 1) * 128],
                            xp[:, c * CH + k : c * CH + k + CH],
                            start=(k == 0),
                            stop=(k == 2),
                        )
                    if c % 2 == 0:
                        nc.vector.tensor_scalar(
                            oc, ps[:, :], bias[:, 0:1], None, mybir.AluOpType.add
                        )
                    else:
                        nc.scalar.activation(
                            oc,
                            ps[:, :],
                            mybir.ActivationFunctionType.Identity,
                            bias=bias[:, 0:1],
                            scale=1.0,
                        )
                if not last:
                    continue
                # last row-tile: fine-grained stores, final two on the
                # (now idle) sync ring, to shorten the tail
                if c == 3:
                    nc.scalar.dma_start(o_d[t][:, 0:HALF], ot[:, 0:HALF])
                elif c == 5:
                    nc.scalar.dma_start(o_d[t][:, HALF : 6 * CH], ot[:, HALF : 6 * CH])
                elif c == 6:
                    nc.sync.dma_start(
                        o_d[t][:, 6 * CH : 7 * CH], ot[:, 6 * CH : 7 * CH]
                    )
                elif c == 7:
                    nc.sync.dma_start(o_d[t][:, 7 * CH : L], ot[:, 7 * CH : L])
            if not last:
                if t == NT - 2:
                    pending_sync_store = (o_d[t], ot[:, :])
                else:
                    nc.scalar.dma_start(o_d[t], ot[:, :])

    nc.compile()
    return nc


def _pack_weights(w, b):
    """Per channel block: 3 diag [128,128] f16 stacked -> [128, 384], plus
    the f32 bias column [128, 1]."""
    w = np.asarray(w, dtype=np.float32).reshape(C, K)
    b = np.asarray(b, dtype=np.float32)
    wds, w1s, biases = [], [], []
    idx = np.arange(128)
    for blk in range(NBLK):
        wblk = w[blk * 128 : (blk + 1) * 128]
        wd = np.zeros((128, 3 * 128), np.float16)
        for k in range(3):
            wd[idx, k * 128 + idx] = wblk[:, k].astype(np.float16)
        wds.append(wd)
        w1s.append(np.ascontiguousarray(wblk[:, 1].reshape(128, 1)))
        biases.append(b[blk * 128 : (blk + 1) * 128].reshape(128, 1))
    return wds, w1s, biases


def _run(inputs, trace=False, **kw):
    from concourse.bass_utils import run_bass_kernel_spmd

    if "nc" not in _STATE:
        _STATE["nc"] = _build_program()
    nc = _STATE["nc"]

    x = np.asarray(inputs["x"], dtype=np.float32).astype(np.float16)
    xpad = np.zeros((B, C, LP), np.float16)
    xpad[:, :, 1 : L + 1] = x
    wds, w1s, biases = _pack_weights(inputs["w"], inputs["b"])
    in_maps = []
    for core in range(N_CORES):
        blk = core % NBLK
        g = core // NBLK
        shard = np.ascontiguousarray(
            xpad[g * B_SH : (g + 1) * B_SH, blk * 128 : (blk + 1) * 128, :]
        )
        in_maps.append(
            {"x": shard, "wd": wds[blk], "w1col": w1s[blk], "bias": biases[blk]}
        )
    res = run_bass_kernel_spmd(
        nc, in_maps, core_ids=list(range(N_CORES)), trace=trace, **kw
    )
    out = np.empty((B, C, L), np.float32)
    for core in range(N_CORES):
        blk = core % NBLK
        g = core // NBLK
        out[g * B_SH : (g + 1) * B_SH, blk * 128 : (blk + 1) * 128, :] = res.results[
            core
        ]["out"].astype(np.float32)
    return out, res


def kernel(**inputs):
    return _run(inputs)[0]
